# revision 3
# baseline (speedup 1.0000x reference)
"""Trainium2 Bass kernel for the windowed 3-channel MLP (dense_mlp).

Reference computation (B=8192):
  x [B, 6144] -> view [B, 3, 2048]
  16 overlapping windows/channel (len 256, stride 119)
  h[b,c,w,:] = win @ W1[c,w] + b1[c,w]          # [B,3,16,64]
  h = mean over c                               # [B,16,64]
  g[b,grp]   = h-grp(4 windows=256) @ W2[grp] + b2   # [B,4,64]
  out        = g.reshape(B,256) @ W3 + b3       # [B,255]

Strategy: pure data parallelism over 8 cores (B/8 = 1024 rows each).
Compute in fp16 (accumulation in f32 PSUM); x is shipped to device DRAM as
fp16 so the DMA-transpose (xbar) can read it directly from DRAM into
feature-major SBUF tiles — no on-chip staging, casting, or SBUF->SBUF pass.

On-device per core:
  - 4 batch chunks (256/384/256/128 rows; small last chunk shrinks the serial
    tail); per chunk one DRAM->SBUF xbar transpose produces
    xT [128k, 48 ktiles, nb].
  - Layer 1 as banded matmuls over 128-aligned k-tiles with host-packed
    zero-padded weight blocks (channel-mean folded into PSUM accumulation,
    1/3 folded into W1).
  - Layers 2/3 stay feature-major; layer 3 uses gT as lhsT so the output
    comes out batch-major for a contiguous DMA out.
"""

import sys

sys.path.insert(0, "/opt/trn_rl_repo")

import numpy as np

import concourse.bass as bass
import concourse.mybir as mybir
import concourse.tile as tile
from concourse import bacc
from concourse.bass_utils import run_bass_kernel_spmd

P = 128
N_CORES = 8
B_FULL = 8192
B_SHARD = B_FULL // N_CORES          # 1024
CH_LEN = 2048
N_CH = 3
K_FULL = N_CH * CH_LEN               # 6144
N_WIN = 16
WIN = 256
STRIDE = 119
N_PAIR = 8                           # window pairs (2 windows x 64 = 128 feats)
KT_CH = CH_LEN // P                  # 16 k-tiles per channel
KT_ALL = K_FULL // P                 # 48
NB = 384                             # max batch chunk (matmul free dim)
CHUNKS = [256, 384, 256, 128]        # batch chunk sizes (sum = B_SHARD)
assert sum(CHUNKS) == B_SHARD
N_OUT = 255

def _pair_tiles(m):
    """k-tiles of one channel that intersect window pair m (rows 238m..238m+374)."""
    lo = (2 * STRIDE * m) // P
    hi = (2 * STRIDE * m + 2 * STRIDE + WIN - 2 - STRIDE) // P  # (238m+374)//128
    return list(range(lo, min(hi, KT_CH - 1) + 1))

# Block order for layer-1 packed weights: for m, for c, for t.
BLOCKS = [(m, c, t) for m in range(N_PAIR) for c in range(N_CH) for t in _pair_tiles(m)]
BLK_IDX = {key: i for i, key in enumerate(BLOCKS)}
N_BLK = len(BLOCKS)                  # 90


def _pack_weights(W1, b1, W2, b2, W3, b3):
    """Host-side packing of the tiny weight tensors into device layouts."""
    W1 = np.asarray(W1, dtype=np.float32)
    ki = np.arange(P)[:, None]                    # tile-local k row
    j = np.arange(P)[None, :]                     # pair-local output feature
    w_off = j // 64                               # window within pair
    n = j % 64

    w1p = np.zeros((N_BLK, P, P), dtype=np.float32)
    for i, (m, c, t) in enumerate(BLOCKS):
        w = 2 * m + w_off                         # [1,128] window index
        koff = P * t + ki - STRIDE * w            # [128,128] k within window
        mask = (koff >= 0) & (koff < WIN)
        w1p[i] = np.where(
            mask, W1[c, w, np.clip(koff, 0, WIN - 1), n] / 3.0, 0.0
        )
    # device layout: [P(ki), N_BLK * P(j)] contiguous per partition
    w1sb = np.ascontiguousarray(
        w1p.transpose(1, 0, 2).reshape(P, N_BLK * P)
    ).astype(np.float16)

    # W2 [4,256,64] -> pieces [g,p][128,64] -> [P, 8, 64]
    w2p = np.asarray(W2, dtype=np.float32).reshape(4, 2, P, 64)
    w2sb = np.ascontiguousarray(
        w2p.transpose(2, 0, 1, 3).reshape(P, 8 * 64)
    ).astype(np.float16)

    # W3 [256,255] -> [P, 2, 255]
    w3p = np.asarray(W3, dtype=np.float32).reshape(2, P, N_OUT)
    w3sb = np.ascontiguousarray(
        w3p.transpose(1, 0, 2).reshape(P, 2 * N_OUT)
    ).astype(np.float16)

    # biases (per-partition layouts)
    b1m = np.asarray(b1, dtype=np.float32).mean(axis=0)        # [16,64]
    b1t = np.ascontiguousarray(b1m.reshape(N_PAIR, P).T)       # [128, 8]
    b2t = np.ascontiguousarray(np.asarray(b2, dtype=np.float32).T)  # [64, 4]
    b3t = np.ascontiguousarray(
        np.broadcast_to(np.asarray(b3, dtype=np.float32), (P, N_OUT))
    )                                                          # [128, 255]
    return w1sb, w2sb, w3sb, b1t, b2t, b3t


def build_kernel(reps=1, has_bias=False, unroll=1):
    nc = bacc.Bacc("TRN2", target_bir_lowering=False, debug=False,
                   num_devices=N_CORES)
    f16 = mybir.dt.float16
    f32 = mybir.dt.float32

    x_ext = nc.declare_dram_parameter("x", [B_SHARD, K_FULL], f16, isOutput=False)
    w1_ext = nc.declare_dram_parameter("w1", [P, N_BLK * P], f16, isOutput=False)
    w2_ext = nc.declare_dram_parameter("w2", [P, 8 * 64], f16, isOutput=False)
    w3_ext = nc.declare_dram_parameter("w3", [P, 2 * N_OUT], f16, isOutput=False)
    b1_ext = nc.declare_dram_parameter("b1t", [P, N_PAIR], f32, isOutput=False)
    b2_ext = nc.declare_dram_parameter("b2t", [64, 4], f32, isOutput=False)
    b3_ext = nc.declare_dram_parameter("b3t", [P, N_OUT], f32, isOutput=False)
    out_ext = nc.declare_dram_parameter("out", [B_SHARD, N_OUT], f32, isOutput=True)

    with tile.TileContext(nc) as tc:
        with (
            tc.tile_pool(name="wpool", bufs=1) as wpool,
            tc.tile_pool(name="xt", bufs=3) as xt_pool,
            tc.tile_pool(name="hp", bufs=10) as hp_pool,
            tc.tile_pool(name="gt", bufs=2) as gt_pool,
            tc.tile_pool(name="osb", bufs=3) as out_pool,
            tc.tile_pool(name="ps1", bufs=4, space="PSUM") as ps1_pool,
            tc.tile_pool(name="ps2", bufs=2, space="PSUM") as ps2_pool,
            tc.tile_pool(name="ps3", bufs=2, space="PSUM") as ps3_pool,
        ):
            w1sb = wpool.tile([P, N_BLK, P], f16)
            nc.scalar.dma_start(out=w1sb[:], in_=w1_ext.rearrange("p (b j) -> p b j", j=P))
            w2sb = wpool.tile([P, 8, 64], f16)
            nc.scalar.dma_start(out=w2sb[:], in_=w2_ext.rearrange("p (b j) -> p b j", j=64))
            w3sb = wpool.tile([P, 2, N_OUT], f16)
            nc.scalar.dma_start(out=w3sb[:], in_=w3_ext.rearrange("p (b j) -> p b j", j=N_OUT))
            b1sb = wpool.tile([P, N_PAIR], f32)
            nc.scalar.dma_start(out=b1sb[:], in_=b1_ext[:])
            b2sb = wpool.tile([64, 4], f32)
            nc.scalar.dma_start(out=b2sb[:], in_=b2_ext[:])
            b3sb = wpool.tile([P, N_OUT], f32)
            nc.scalar.dma_start(out=b3sb[:], in_=b3_ext[:])

            import contextlib
            loop_cm = tc.For_i(0, reps, 1) if reps > 1 else contextlib.nullcontext()
            with loop_cm:
                for _ in range(unroll):
                    _kernel_body(nc, tc, locals(), has_bias)

    nc.compile()
    return nc


def _kernel_body(nc, tc, env, has_bias):
    x_ext = env["x_ext"]
    out_ext = env["out_ext"]
    w1sb, w2sb, w3sb = env["w1sb"], env["w2sb"], env["w3sb"]
    b1sb, b2sb, b3sb = env["b1sb"], env["b2sb"], env["b3sb"]
    xt_pool = env["xt_pool"]
    hp_pool, gt_pool, out_pool = env["hp_pool"], env["gt_pool"], env["out_pool"]
    ps1_pool, ps2_pool, ps3_pool = env["ps1_pool"], env["ps2_pool"], env["ps3_pool"]
    f16 = mybir.dt.float16
    f32 = mybir.dt.float32

    b0 = 0
    for ch, nb in enumerate(CHUNKS):
        # one xbar transpose: x[b0:b0+nb, :] (DRAM, fp16) -> [128k, 48, nb]
        xt_t = xt_pool.tile([P, KT_ALL, NB], f16, name="xtt")
        xt = xt_t[:, :, :nb]
        nc.sync.dma_start(out=xt[:], in_=x_ext[b0:b0 + nb, :], transpose=True)

        # ---- layer 1: banded matmuls per window pair ----
        hps = {}
        for m in range(N_PAIR):
            ps_t = ps1_pool.tile([P, NB], f32, name="ps1t")
            ps = ps_t[:, :nb]
            mm_list = [(c, t) for c in range(N_CH) for t in _pair_tiles(m)]
            for i, (c, t) in enumerate(mm_list):
                nc.tensor.matmul(
                    ps[:],
                    w1sb[:, BLK_IDX[(m, c, t)], :],
                    xt[:, c * KT_CH + t, :],
                    start=(i == 0),
                    stop=(i == len(mm_list) - 1),
                )
            hp_t = hp_pool.tile([P, NB], f16, name="hpt")
            hp = hp_t[:, :nb]
            if has_bias:
                nc.vector.tensor_scalar_add(hp[:], ps[:], b1sb[:, m:m + 1])
            else:
                nc.vector.tensor_copy(out=hp[:], in_=ps[:])
            hps[m] = hp

        # ---- layer 2: 4 groups of 4 windows ----
        gt_t = gt_pool.tile([P, 2, NB], f16, name="gtt")
        gt = gt_t[:, :, :nb]
        for g in range(4):
            ps2_t = ps2_pool.tile([64, NB], f32, name="ps2t")
            ps2 = ps2_t[:, :nb]
            for piece in range(2):
                nc.tensor.matmul(
                    ps2[:],
                    w2sb[:, 2 * g + piece, :],
                    hps[2 * g + piece][:],
                    start=(piece == 0),
                    stop=(piece == 1),
                )
            lo = 64 * (g % 2)
            if has_bias:
                nc.vector.tensor_scalar_add(
                    gt[lo:lo + 64, g // 2], ps2[:], b2sb[:, g:g + 1],
                )
            else:
                nc.vector.tensor_copy(out=gt[lo:lo + 64, g // 2], in_=ps2[:])

        # ---- layer 3: back to batch-major ----
        osb_t = out_pool.tile([P, NB // P, N_OUT], f32, name="osbt")
        osb = osb_t[:, :nb // P]
        for js in range(nb // P):
            ps3 = ps3_pool.tile([P, N_OUT], f32)
            for piece in range(2):
                nc.tensor.matmul(
                    ps3[:],
                    gt[:, piece, js * P:(js + 1) * P],
                    w3sb[:, piece, :],
                    start=(piece == 0),
                    stop=(piece == 1),
                )
            if has_bias:
                nc.vector.tensor_tensor(
                    osb[:, js], ps3[:], b3sb[:], mybir.AluOpType.add,
                )
            else:
                nc.vector.tensor_copy(out=osb[:, js], in_=ps3[:])
        nc.scalar.dma_start(
            out=out_ext[b0:b0 + nb, :].rearrange("(j p) n -> p j n", p=P),
            in_=osb[:],
        )
        b0 += nb


_CACHED_NC = None


def _prep_in_maps(x, W1, b1, W2, b2, W3, b3):
    x16 = np.asarray(x, dtype=np.float16)
    w1sb, w2sb, w3sb, b1t, b2t, b3t = _pack_weights(W1, b1, W2, b2, W3, b3)
    in_maps = []
    for i in range(N_CORES):
        in_maps.append({
            "x": x16[i * B_SHARD:(i + 1) * B_SHARD],
            "w1": w1sb,
            "w2": w2sb,
            "w3": w3sb,
            "b1t": b1t,
            "b2t": b2t,
            "b3t": b3t,
        })
    return in_maps


_CACHED_BIAS_NC = None


def kernel(x, W1, b1, W2, b2, W3, b3):
    global _CACHED_NC, _CACHED_BIAS_NC
    has_bias = bool(
        np.any(np.asarray(b1)) or np.any(np.asarray(b2)) or np.any(np.asarray(b3))
    )
    if has_bias:
        if _CACHED_BIAS_NC is None:
            _CACHED_BIAS_NC = build_kernel(has_bias=True)
        nc = _CACHED_BIAS_NC
    else:
        if _CACHED_NC is None:
            _CACHED_NC = build_kernel()
        nc = _CACHED_NC
    in_maps = _prep_in_maps(x, W1, b1, W2, b2, W3, b3)
    last_err = None
    for attempt in range(3):
        try:
            res = run_bass_kernel_spmd(nc, in_maps, core_ids=list(range(N_CORES)))
            break
        except Exception as e:  # transient device/axon failures
            last_err = e
            if attempt == 2:
                raise
            import time as _time
            _time.sleep(20.0)
    return np.concatenate([res.results[i]["out"] for i in range(N_CORES)], axis=0)



# revision 34
# speedup vs baseline: 6.1131x; 6.1131x over previous
"""Trainium2 Bass kernel for the windowed 3-channel MLP (dense_mlp).

Reference computation (B=8192):
  x [B, 6144] -> view [B, 3, 2048]
  16 overlapping windows/channel (len 256, stride 119)
  h[b,c,w,:] = win @ W1[c,w] + b1[c,w]          # [B,3,16,64]
  h = mean over c                               # [B,16,64]
  g[b,grp]   = h-grp(4 windows=256) @ W2[grp] + b2   # [B,4,64]
  out        = g.reshape(B,256) @ W3 + b3       # [B,255]

Strategy: pure data parallelism over 8 cores (B/8 = 1024 rows each), fp16
compute with f32 PSUM accumulation.

Key measured facts driving the design (single-core loop-marginal timing):
  - The DMA xbar transpose runs at only ~164 GB/s and bound the old kernel,
    so x is pre-transposed on the HOST into a feature-major, chunk-packed
    fp16 layout ([128 part, 48*nb] per chunk, one contiguous run per
    partition) and loaded with plain ~350 GB/s DMAs. A single HWDGE queue
    already saturates HBM; multi-queue/dual-ring variants measured slower.
  - Matmuls carry a ~10-40 ns fixed cost, so layer-1 streams the widest
    PSUM-legal free dim (N=512) per chunk.
  - Loads are split per channel (`csplit`): an L1 matmul never spans
    channels, so 3 smaller tiles+DMAs per chunk pipeline loads against
    compute without shrinking matmul width.
  - Layers 2/3 of chunk c are emitted after chunk c+1's layer-1 matmuls
    (`defer_l23`) so the PE never waits on DVE PSUM->SBUF copies.

On-device per core per iteration:
  - per chunk: 3 channel loads -> 90 banded L1 matmuls into 8 pair-PSUMs
    (channel-mean folded into accumulation, 1/3 into W1) -> DVE copies to
    fp16 -> L2 (4 groups, paired into 128-wide PSUM) -> L3 with gT as lhsT
    so the output is batch-major for a contiguous gpsimd (SWDGE) store.
"""

import sys

sys.path.insert(0, "/opt/trn_rl_repo")

import numpy as np

import concourse.bass as bass
import concourse.mybir as mybir
import concourse.tile as tile
from concourse import bacc
from concourse.bass_utils import run_bass_kernel_spmd

P = 128
N_CORES = 8
B_FULL = 8192
B_SHARD = B_FULL // N_CORES          # 1024
CH_LEN = 2048
N_CH = 3
K_FULL = N_CH * CH_LEN               # 6144
N_WIN = 16
WIN = 256
STRIDE = 119
N_PAIR = 8                           # window pairs (2 windows x 64 = 128 feats)
KT_CH = CH_LEN // P                  # 16 k-tiles per channel
KT_ALL = K_FULL // P                 # 48
NB = 384                             # max batch chunk (matmul free dim)
CHUNKS = [128, 384, 384, 128]        # batch chunk sizes (sum = B_SHARD)
assert sum(CHUNKS) == B_SHARD
N_OUT = 255

def _pair_tiles(m):
    """k-tiles of one channel that intersect window pair m (rows 238m..238m+374)."""
    lo = (2 * STRIDE * m) // P
    hi = (2 * STRIDE * m + 2 * STRIDE + WIN - 2 - STRIDE) // P  # (238m+374)//128
    return list(range(lo, min(hi, KT_CH - 1) + 1))

# Block order for layer-1 packed weights: for m, for c, for t.
BLOCKS = [(m, c, t) for m in range(N_PAIR) for c in range(N_CH) for t in _pair_tiles(m)]
BLK_IDX = {key: i for i, key in enumerate(BLOCKS)}
N_BLK = len(BLOCKS)                  # 90


def _pack_weights(W1, b1, W2, b2, W3, b3):
    """Host-side packing of the tiny weight tensors into device layouts."""
    W1 = np.asarray(W1, dtype=np.float32)
    ki = np.arange(P)[:, None]                    # tile-local k row
    j = np.arange(P)[None, :]                     # pair-local output feature
    w_off = j // 64                               # window within pair
    n = j % 64

    w1p = np.zeros((N_BLK, P, P), dtype=np.float32)
    for i, (m, c, t) in enumerate(BLOCKS):
        w = 2 * m + w_off                         # [1,128] window index
        koff = P * t + ki - STRIDE * w            # [128,128] k within window
        mask = (koff >= 0) & (koff < WIN)
        w1p[i] = np.where(
            mask, W1[c, w, np.clip(koff, 0, WIN - 1), n] / 3.0, 0.0
        )
    # device layout: [P(ki), N_BLK * P(j)] contiguous per partition
    w1sb = np.ascontiguousarray(
        w1p.transpose(1, 0, 2).reshape(P, N_BLK * P)
    ).astype(np.float16)

    # W2 [4,256,64] -> pieces [g,p][128,64] -> [P, 8, 64]
    w2p = np.asarray(W2, dtype=np.float32).reshape(4, 2, P, 64)
    w2sb = np.ascontiguousarray(
        w2p.transpose(2, 0, 1, 3).reshape(P, 8 * 64)
    ).astype(np.float16)

    # W3 [256,255] -> [P, 2, 255]
    w3p = np.asarray(W3, dtype=np.float32).reshape(2, P, N_OUT)
    w3sb = np.ascontiguousarray(
        w3p.transpose(1, 0, 2).reshape(P, 2 * N_OUT)
    ).astype(np.float16)

    # biases (per-partition layouts)
    b1m = np.asarray(b1, dtype=np.float32).mean(axis=0)        # [16,64]
    b1t = np.ascontiguousarray(b1m.reshape(N_PAIR, P).T)       # [128, 8]
    b2t = np.ascontiguousarray(np.asarray(b2, dtype=np.float32).T)  # [64, 4]
    b3t = np.ascontiguousarray(
        np.broadcast_to(np.asarray(b3, dtype=np.float32), (P, N_OUT))
    )                                                          # [128, 255]
    return w1sb, w2sb, w3sb, b1t, b2t, b3t


def build_kernel(reps=1, has_bias=False, unroll=1, opts=None):
    nc = bacc.Bacc("TRN2", target_bir_lowering=False, debug=False,
                   num_devices=N_CORES)
    f16 = mybir.dt.float16
    f32 = mybir.dt.float32

    if opts and opts.get("host_t"):
        # host pre-transposed, chunk-packed: [128, sum(48*nb)] (see
        # _pack_x_host) so each chunk load is one contiguous run/partition
        x_ext = nc.declare_dram_parameter("x", [P, KT_ALL * B_SHARD], f16,
                                          isOutput=False)
    else:
        x_ext = nc.declare_dram_parameter("x", [B_SHARD, K_FULL], f16,
                                          isOutput=False)
    w1_ext = nc.declare_dram_parameter("w1", [P, N_BLK * P], f16, isOutput=False)
    w2_ext = nc.declare_dram_parameter("w2", [P, 8 * 64], f16, isOutput=False)
    w3_ext = nc.declare_dram_parameter("w3", [P, 2 * N_OUT], f16, isOutput=False)
    b1_ext = nc.declare_dram_parameter("b1t", [P, N_PAIR], f32, isOutput=False)
    b2_ext = nc.declare_dram_parameter("b2t", [64, 4], f32, isOutput=False)
    b3_ext = nc.declare_dram_parameter("b3t", [P, N_OUT], f32, isOutput=False)
    out_ext = nc.declare_dram_parameter("out", [B_SHARD, N_OUT], f32, isOutput=True)

    with tile.TileContext(nc) as tc:
        with (
            tc.tile_pool(name="wpool", bufs=1) as wpool,
            tc.tile_pool(name="xt", bufs=1) as xt_pool,
            tc.tile_pool(
                name="hp",
                bufs=(18 if opts and opts.get("defer_l23") else 10),
            ) as hp_pool,
            tc.tile_pool(name="gt", bufs=2) as gt_pool,
            tc.tile_pool(name="osb", bufs=1) as out_pool,
            tc.tile_pool(name="ps1", bufs=(opts or {}).get("ps1_bufs", 4), space="PSUM") as ps1_pool,
            tc.tile_pool(name="ps2", bufs=(opts or {}).get("ps2_bufs", 2), space="PSUM") as ps2_pool,
            tc.tile_pool(name="ps3", bufs=(opts or {}).get("ps3_bufs", 2), space="PSUM") as ps3_pool,
        ):
            w1sb = wpool.tile([P, N_BLK, P], f16)
            nc.scalar.dma_start(out=w1sb[:], in_=w1_ext.rearrange("p (b j) -> p b j", j=P))
            w2sb = wpool.tile([P, 8, 64], f16)
            nc.scalar.dma_start(out=w2sb[:], in_=w2_ext.rearrange("p (b j) -> p b j", j=64))
            w3sb = wpool.tile([P, 2, N_OUT], f16)
            nc.scalar.dma_start(out=w3sb[:], in_=w3_ext.rearrange("p (b j) -> p b j", j=N_OUT))
            b1sb = wpool.tile([P, N_PAIR], f32)
            nc.scalar.dma_start(out=b1sb[:], in_=b1_ext[:])
            b2sb = wpool.tile([64, 4], f32)
            nc.scalar.dma_start(out=b2sb[:], in_=b2_ext[:])
            b3sb = wpool.tile([P, N_OUT], f32)
            nc.scalar.dma_start(out=b3sb[:], in_=b3_ext[:])

            static_xt = None
            if opts and opts.get("no_dma"):
                # micro-bench: load x once before the loop, body is compute-only
                static_xt = []
                b0 = 0
                for ch, nb in enumerate(opts.get("chunks", CHUNKS)):
                    sxt = wpool.tile([P, KT_ALL, nb], f16, name=f"sxt{ch}")
                    if opts.get("host_t"):
                        nc.sync.dma_start(
                            out=sxt[:],
                            in_=x_ext[:, KT_ALL * b0:KT_ALL * (b0 + nb)]
                            .rearrange("p (t b) -> p t b", b=nb))
                    else:
                        nc.sync.dma_start(out=sxt[:], in_=x_ext[b0:b0 + nb, :],
                                          transpose=True)
                    static_xt.append(sxt)
                    b0 += nb

            import contextlib
            loop_cm = tc.For_i(0, reps, 1) if reps > 1 else contextlib.nullcontext()
            with loop_cm:
                for _ in range(unroll):
                    _kernel_body(nc, tc, locals(), has_bias, opts or {})

    nc.compile()
    return nc


def _kernel_body(nc, tc, env, has_bias, opts=None):
    opts = opts or {}
    chunks = opts.get("chunks", CHUNKS)
    ring_mode = opts.get("rings", "dual")
    out_eng = opts.get("out_eng", "gpsimd")
    no_compute = opts.get("no_compute", False)   # micro-bench: DMAs only
    no_dma = opts.get("no_dma", False)           # micro-bench: compute only
    env["opts"] = opts
    defer_l23 = opts.get("defer_l23", False)
    pending = None
    env["nbmax"] = max(chunks)
    x_ext = env["x_ext"]
    out_ext = env["out_ext"]
    w1sb, w2sb, w3sb = env["w1sb"], env["w2sb"], env["w3sb"]
    b1sb, b2sb, b3sb = env["b1sb"], env["b2sb"], env["b3sb"]
    xt_pool = env["xt_pool"]
    hp_pool, gt_pool, out_pool = env["hp_pool"], env["gt_pool"], env["out_pool"]
    ps1_pool, ps2_pool, ps3_pool = env["ps1_pool"], env["ps2_pool"], env["ps3_pool"]
    f16 = mybir.dt.float16
    f32 = mybir.dt.float32

    b0 = 0
    for ch, nb in enumerate(chunks):
        # one xbar transpose: x[b0:b0+nb, :] (DRAM, fp16) -> [128k, 48, nb].
        # Exact-size tile => contiguous SBUF destination; optionally alternate
        # the two HWDGE rings (SP / ACT) so two transposes can be in flight.
        engs = {"sync": nc.sync, "scalar": nc.scalar, "gpsimd": nc.gpsimd}
        load_rings = opts.get(
            "load_rings",
            ["sync"] if ring_mode == "single" else ["sync", "scalar"])
        ring = engs[load_rings[ch % len(load_rings)]]
        xt_parts = None
        if env.get("static_xt"):
            xt = env["static_xt"][ch]
        elif opts.get("host_t") and opts.get("csplit"):
            # one tile + one DMA per channel (or half-channel with csplit=2):
            # finer load granularity while keeping full-width matmuls (an L1
            # matmul never spans channels)
            sub = 2 if opts.get("csplit") == 2 else 1
            kt_piece = KT_CH // sub
            xt_parts = []
            for c in range(N_CH * sub):
                xc = xt_pool.tile([P, kt_piece, nb], f16, name=f"xtc{ch}c{c}")
                off = KT_ALL * b0 + c * kt_piece * nb
                ring.dma_start(
                    out=xc[:],
                    in_=x_ext[:, off:off + kt_piece * nb]
                    .rearrange("p (t b) -> p t b", b=nb))
                xt_parts.append(xc)
            env["kt_piece"] = kt_piece
            xt = None
        else:
            xt_t = xt_pool.tile([P, KT_ALL, nb], f16, name=f"xtc{ch}")
            xt = xt_t
            if opts.get("host_t"):
                ring.dma_start(
                    out=xt[:],
                    in_=x_ext[:, KT_ALL * b0:KT_ALL * (b0 + nb)]
                    .rearrange("p (t b) -> p t b", b=nb))
            else:
                ring.dma_start(out=xt[:], in_=x_ext[b0:b0 + nb, :],
                               transpose=True)

        if no_compute:
            # anti-DCE consumer: one tiny matmul + copy, then store
            xt0 = xt_parts[0][:, 0, :] if xt_parts is not None else xt[:, 0, :]
            ps_t = ps1_pool.tile([P, nb], f32, name="ncps")
            nc.tensor.matmul(ps_t[:], w1sb[:, 0, :], xt0,
                             start=True, stop=True)
            osb_t = out_pool.tile([P, nb // P, N_OUT], f32, name=f"osbc{ch}")
            nc.vector.tensor_copy(out=osb_t[:, 0, :128], in_=ps_t[:, :128])
            oeng = {"gpsimd": nc.gpsimd, "scalar": nc.scalar,
                    "sync": nc.sync}[out_eng]
            oeng.dma_start(
                out=out_ext[b0:b0 + nb, :].rearrange("(j p) n -> p j n", p=P),
                in_=osb_t[:],
            )
            b0 += nb
            continue

        # ---- layer 1: banded matmuls per window pair ----
        hps = {}
        for m in range(N_PAIR):
            ps_t = ps1_pool.tile([P, env["nbmax"]], f32, name="ps1t")
            ps = ps_t[:, :nb]
            mm_list = [(c, t) for c in range(N_CH) for t in _pair_tiles(m)]
            for i, (c, t) in enumerate(mm_list):
                if xt_parts is not None:
                    kt_piece = env["kt_piece"]
                    kg = c * KT_CH + t
                    rhs = xt_parts[kg // kt_piece][:, kg % kt_piece, :]
                else:
                    rhs = xt[:, c * KT_CH + t, :]
                nc.tensor.matmul(
                    ps[:],
                    w1sb[:, BLK_IDX[(m, c, t)], :],
                    rhs,
                    start=(i == 0),
                    stop=(i == len(mm_list) - 1),
                )
            hp_t = hp_pool.tile([P, env["nbmax"]], f16, name="hpt")
            hp = hp_t[:, :nb]
            if has_bias:
                nc.vector.tensor_scalar_add(hp[:], ps[:], b1sb[:, m:m + 1])
            elif opts.get("act_copies") and m % 2 == 1:
                nc.scalar.activation(hp[:], ps[:],
                                     mybir.ActivationFunctionType.Copy)
            else:
                nc.vector.tensor_copy(out=hp[:], in_=ps[:])
            hps[m] = hp

        if opts.get("l1_only"):
            # micro-bench: one tiny consumer per chunk so L1 isn't DCE'd
            osb_t = out_pool.tile([P, nb // P, N_OUT], f32, name=f"osbc{ch}")
            nc.vector.tensor_copy(out=osb_t[:, 0, :128], in_=hps[7][:, :128])
            oeng = {"gpsimd": nc.gpsimd, "scalar": nc.scalar,
                    "sync": nc.sync}[out_eng]
            oeng.dma_start(
                out=out_ext[b0:b0 + nb, :].rearrange("(j p) n -> p j n", p=P),
                in_=osb_t[:],
            )
        elif defer_l23:
            # emit chunk c-1's layers 2/3 AFTER chunk c's layer-1 matmuls so
            # the PE never waits on the DVE copies at a chunk boundary
            if pending is not None:
                _l23(nc, env, has_bias, out_eng, *pending)
            pending = (ch, nb, b0, hps)
        else:
            _l23(nc, env, has_bias, out_eng, ch, nb, b0, hps)
        b0 += nb
    if pending is not None:
        _l23(nc, env, has_bias, out_eng, *pending)


def _l23(nc, env, has_bias, out_eng, ch, nb, b0, hps):
    out_ext = env["out_ext"]
    opts = env.get("opts") or {}
    w2sb, w3sb = env["w2sb"], env["w3sb"]
    b2sb, b3sb = env["b2sb"], env["b3sb"]
    gt_pool, out_pool = env["gt_pool"], env["out_pool"]
    ps2_pool, ps3_pool = env["ps2_pool"], env["ps3_pool"]
    if opts.get("ps23_merge"):
        ps3_pool = ps2_pool
    f16 = mybir.dt.float16
    f32 = mybir.dt.float32

    # ---- layer 2: 4 groups of 4 windows; two 64-wide groups share one
    # 128-partition PSUM tile so the PSUM->SBUF copy is full-width ----
    gt_t = gt_pool.tile([P, 2, env.get("nbmax", NB)], f16, name="gtt")
    gt = gt_t[:, :, :nb]
    for gp in range(2):              # group pair (2*gp, 2*gp+1)
        ps2_t = ps2_pool.tile([P, env.get("nbmax", NB)], f32, name="ps2t")
        ps2 = ps2_t[:, :nb]
        for half in range(2):
            g = 2 * gp + half
            lo = 64 * half
            for piece in range(2):
                nc.tensor.matmul(
                    ps2[lo:lo + 64],
                    w2sb[:, 2 * g + piece, :],
                    hps[2 * g + piece][:],
                    start=(piece == 0),
                    stop=(piece == 1),
                )
        if has_bias:
            for half in range(2):
                g = 2 * gp + half
                lo = 64 * half
                nc.vector.tensor_scalar_add(
                    gt[lo:lo + 64, gp], ps2[lo:lo + 64], b2sb[:, g:g + 1],
                )
        else:
            nc.vector.tensor_copy(out=gt[:, gp], in_=ps2[:])

    # ---- layer 3: back to batch-major ----
    osb_t = out_pool.tile([P, nb // P, N_OUT], f32, name=f"osbc{ch}")
    osb = osb_t
    for js in range(nb // P):
        ps3 = ps3_pool.tile([P, N_OUT], f32, name="ps3t")
        for piece in range(2):
            nc.tensor.matmul(
                ps3[:],
                gt[:, piece, js * P:(js + 1) * P],
                w3sb[:, piece, :],
                start=(piece == 0),
                stop=(piece == 1),
            )
        if has_bias:
            nc.vector.tensor_tensor(
                osb[:, js], ps3[:], b3sb[:], mybir.AluOpType.add,
            )
        else:
            nc.vector.tensor_copy(out=osb[:, js], in_=ps3[:])
    # SWDGE (gpsimd) store keeps both HWDGE rings free for transposes
    oeng = {"gpsimd": nc.gpsimd, "scalar": nc.scalar, "sync": nc.sync}[out_eng]
    oeng.dma_start(
        out=out_ext[b0:b0 + nb, :].rearrange("(j p) n -> p j n", p=P),
        in_=osb[:],
    )


_CACHED_NC = None


def _pack_x_host(x16_shard, chunks, csplit=False):
    """[B_SHARD, 6144] -> [128, 48*B_SHARD]: per chunk (and per channel when
    csplit), k-tile-major, partition-major so each DMA is one contiguous run
    per partition."""
    parts = []
    b0 = 0
    for nb in chunks:
        blk = x16_shard[b0:b0 + nb]                       # [nb, 6144]
        h = blk.T.reshape(KT_ALL, P, nb).transpose(1, 0, 2)   # [128, 48, nb]
        # csplit keeps channel blocks adjacent, which this layout already does
        # (k-tiles 0-15 = ch0, 16-31 = ch1, 32-47 = ch2)
        parts.append(h.reshape(P, KT_ALL * nb))
        b0 += nb
    return np.ascontiguousarray(np.concatenate(parts, axis=1))


def _prep_in_maps(x, W1, b1, W2, b2, W3, b3, opts=None):
    opts = opts or {}
    x16 = np.asarray(x, dtype=np.float16)
    w1sb, w2sb, w3sb, b1t, b2t, b3t = _pack_weights(W1, b1, W2, b2, W3, b3)
    in_maps = []
    for i in range(N_CORES):
        xs = x16[i * B_SHARD:(i + 1) * B_SHARD]
        if opts.get("host_t"):
            xs = _pack_x_host(xs, opts.get("chunks", CHUNKS))
        in_maps.append({
            "x": xs,
            "w1": w1sb,
            "w2": w2sb,
            "w3": w3sb,
            "b1t": b1t,
            "b2t": b2t,
            "b3t": b3t,
        })
    return in_maps


_CACHED_BIAS_NC = None

# best-measured configuration (see abtest batches): host pre-transposed x,
# plain single-queue loads, layers 2/3 deferred past the next chunk's layer 1
BEST_OPTS = {
    "host_t": True,
    "rings": "single",
    "chunks": [512, 512],
    "defer_l23": True,
    "csplit": True,
}
BEST_UNROLL = 6  # For_i bodies per loop iteration in the timing harness


def kernel(x, W1, b1, W2, b2, W3, b3):
    global _CACHED_NC, _CACHED_BIAS_NC
    has_bias = bool(
        np.any(np.asarray(b1)) or np.any(np.asarray(b2)) or np.any(np.asarray(b3))
    )
    if has_bias:
        if _CACHED_BIAS_NC is None:
            _CACHED_BIAS_NC = build_kernel(has_bias=True, opts=BEST_OPTS)
        nc = _CACHED_BIAS_NC
    else:
        if _CACHED_NC is None:
            _CACHED_NC = build_kernel(opts=BEST_OPTS)
        nc = _CACHED_NC
    in_maps = _prep_in_maps(x, W1, b1, W2, b2, W3, b3, opts=BEST_OPTS)
    last_err = None
    for attempt in range(3):
        try:
            res = run_bass_kernel_spmd(nc, in_maps, core_ids=list(range(N_CORES)))
            break
        except Exception as e:  # transient device/axon failures
            last_err = e
            if attempt == 2:
                raise
            import time as _time
            _time.sleep(20.0)
    return np.concatenate([res.results[i]["out"] for i in range(N_CORES)], axis=0)



# revision 36
# speedup vs baseline: 6.1407x; 1.0045x over previous
"""Trainium2 Bass kernel for the windowed 3-channel MLP (dense_mlp).

Reference computation (B=8192):
  x [B, 6144] -> view [B, 3, 2048]
  16 overlapping windows/channel (len 256, stride 119)
  h[b,c,w,:] = win @ W1[c,w] + b1[c,w]          # [B,3,16,64]
  h = mean over c                               # [B,16,64]
  g[b,grp]   = h-grp(4 windows=256) @ W2[grp] + b2   # [B,4,64]
  out        = g.reshape(B,256) @ W3 + b3       # [B,255]

Strategy: pure data parallelism over 8 cores (B/8 = 1024 rows each), fp16
compute with f32 PSUM accumulation.

Key measured facts driving the design (single-core loop-marginal timing):
  - The DMA xbar transpose runs at only ~164 GB/s and bound the old kernel,
    so x is pre-transposed on the HOST into a feature-major, chunk-packed
    fp16 layout ([128 part, 48*nb] per chunk, one contiguous run per
    partition) and loaded with plain ~350 GB/s DMAs. A single HWDGE queue
    already saturates HBM; multi-queue/dual-ring variants measured slower.
  - Matmuls carry a ~10-40 ns fixed cost, so layer-1 streams the widest
    PSUM-legal free dim (N=512) per chunk.
  - Loads are split per channel (`csplit`): an L1 matmul never spans
    channels, so 3 smaller tiles+DMAs per chunk pipeline loads against
    compute without shrinking matmul width.
  - Layers 2/3 of chunk c are emitted after chunk c+1's layer-1 matmuls
    (`defer_l23`) so the PE never waits on DVE PSUM->SBUF copies.

On-device per core per iteration:
  - per chunk: 3 channel loads -> 90 banded L1 matmuls into 8 pair-PSUMs
    (channel-mean folded into accumulation, 1/3 into W1) -> DVE copies to
    fp16 -> L2 (4 groups, paired into 128-wide PSUM) -> L3 with gT as lhsT
    so the output is batch-major for a contiguous gpsimd (SWDGE) store.
"""

import sys

sys.path.insert(0, "/opt/trn_rl_repo")

import numpy as np

import concourse.bass as bass
import concourse.mybir as mybir
import concourse.tile as tile
from concourse import bacc
from concourse.bass_utils import run_bass_kernel_spmd

P = 128
N_CORES = 8
B_FULL = 8192
B_SHARD = B_FULL // N_CORES          # 1024
CH_LEN = 2048
N_CH = 3
K_FULL = N_CH * CH_LEN               # 6144
N_WIN = 16
WIN = 256
STRIDE = 119
N_PAIR = 8                           # window pairs (2 windows x 64 = 128 feats)
KT_CH = CH_LEN // P                  # 16 k-tiles per channel
KT_ALL = K_FULL // P                 # 48
NB = 384                             # max batch chunk (matmul free dim)
CHUNKS = [128, 384, 384, 128]        # batch chunk sizes (sum = B_SHARD)
assert sum(CHUNKS) == B_SHARD
N_OUT = 255

def _pair_tiles(m):
    """k-tiles of one channel that intersect window pair m (rows 238m..238m+374)."""
    lo = (2 * STRIDE * m) // P
    hi = (2 * STRIDE * m + 2 * STRIDE + WIN - 2 - STRIDE) // P  # (238m+374)//128
    return list(range(lo, min(hi, KT_CH - 1) + 1))

# Block order for layer-1 packed weights: for m, for c, for t.
BLOCKS = [(m, c, t) for m in range(N_PAIR) for c in range(N_CH) for t in _pair_tiles(m)]
BLK_IDX = {key: i for i, key in enumerate(BLOCKS)}
N_BLK = len(BLOCKS)                  # 90


def _pack_weights(W1, b1, W2, b2, W3, b3):
    """Host-side packing of the tiny weight tensors into device layouts."""
    W1 = np.asarray(W1, dtype=np.float32)
    ki = np.arange(P)[:, None]                    # tile-local k row
    j = np.arange(P)[None, :]                     # pair-local output feature
    w_off = j // 64                               # window within pair
    n = j % 64

    w1p = np.zeros((N_BLK, P, P), dtype=np.float32)
    for i, (m, c, t) in enumerate(BLOCKS):
        w = 2 * m + w_off                         # [1,128] window index
        koff = P * t + ki - STRIDE * w            # [128,128] k within window
        mask = (koff >= 0) & (koff < WIN)
        w1p[i] = np.where(
            mask, W1[c, w, np.clip(koff, 0, WIN - 1), n] / 3.0, 0.0
        )
    # device layout: [P(ki), N_BLK * P(j)] contiguous per partition
    w1sb = np.ascontiguousarray(
        w1p.transpose(1, 0, 2).reshape(P, N_BLK * P)
    ).astype(np.float16)

    # W2 [4,256,64] -> pieces [g,p][128,64] -> [P, 8, 64]
    w2p = np.asarray(W2, dtype=np.float32).reshape(4, 2, P, 64)
    w2sb = np.ascontiguousarray(
        w2p.transpose(2, 0, 1, 3).reshape(P, 8 * 64)
    ).astype(np.float16)

    # W3 [256,255] -> [P, 2, 255]
    w3p = np.asarray(W3, dtype=np.float32).reshape(2, P, N_OUT)
    w3sb = np.ascontiguousarray(
        w3p.transpose(1, 0, 2).reshape(P, 2 * N_OUT)
    ).astype(np.float16)

    # biases (per-partition layouts)
    b1m = np.asarray(b1, dtype=np.float32).mean(axis=0)        # [16,64]
    b1t = np.ascontiguousarray(b1m.reshape(N_PAIR, P).T)       # [128, 8]
    b2t = np.ascontiguousarray(np.asarray(b2, dtype=np.float32).T)  # [64, 4]
    b3t = np.ascontiguousarray(
        np.broadcast_to(np.asarray(b3, dtype=np.float32), (P, N_OUT))
    )                                                          # [128, 255]
    return w1sb, w2sb, w3sb, b1t, b2t, b3t


def build_kernel(reps=1, has_bias=False, unroll=1, opts=None):
    nc = bacc.Bacc("TRN2", target_bir_lowering=False, debug=False,
                   num_devices=N_CORES)
    f16 = mybir.dt.float16
    f32 = mybir.dt.float32

    if opts and opts.get("host_t"):
        # host pre-transposed, chunk-packed: [128, sum(48*nb)] (see
        # _pack_x_host) so each chunk load is one contiguous run/partition
        x_ext = nc.declare_dram_parameter("x", [P, KT_ALL * B_SHARD], f16,
                                          isOutput=False)
    else:
        x_ext = nc.declare_dram_parameter("x", [B_SHARD, K_FULL], f16,
                                          isOutput=False)
    w1_ext = nc.declare_dram_parameter("w1", [P, N_BLK * P], f16, isOutput=False)
    w2_ext = nc.declare_dram_parameter("w2", [P, 8 * 64], f16, isOutput=False)
    w3_ext = nc.declare_dram_parameter("w3", [P, 2 * N_OUT], f16, isOutput=False)
    b1_ext = nc.declare_dram_parameter("b1t", [P, N_PAIR], f32, isOutput=False)
    b2_ext = nc.declare_dram_parameter("b2t", [64, 4], f32, isOutput=False)
    b3_ext = nc.declare_dram_parameter("b3t", [P, N_OUT], f32, isOutput=False)
    out_ext = nc.declare_dram_parameter("out", [B_SHARD, N_OUT], f32, isOutput=True)

    with tile.TileContext(nc) as tc:
        with (
            tc.tile_pool(name="wpool", bufs=1) as wpool,
            tc.tile_pool(name="xt", bufs=1) as xt_pool,
            tc.tile_pool(
                name="hp",
                bufs=(26 if opts and opts.get("defer_l23") == 2
                      else 18 if opts and opts.get("defer_l23") else 10),
            ) as hp_pool,
            tc.tile_pool(name="gt", bufs=2) as gt_pool,
            tc.tile_pool(name="osb", bufs=1) as out_pool,
            tc.tile_pool(name="ps1", bufs=(opts or {}).get("ps1_bufs", 4), space="PSUM") as ps1_pool,
            tc.tile_pool(name="ps2", bufs=(opts or {}).get("ps2_bufs", 2), space="PSUM") as ps2_pool,
            tc.tile_pool(name="ps3", bufs=(opts or {}).get("ps3_bufs", 2), space="PSUM") as ps3_pool,
        ):
            w1sb = wpool.tile([P, N_BLK, P], f16)
            nc.scalar.dma_start(out=w1sb[:], in_=w1_ext.rearrange("p (b j) -> p b j", j=P))
            w2sb = wpool.tile([P, 8, 64], f16)
            nc.scalar.dma_start(out=w2sb[:], in_=w2_ext.rearrange("p (b j) -> p b j", j=64))
            w3sb = wpool.tile([P, 2, N_OUT], f16)
            nc.scalar.dma_start(out=w3sb[:], in_=w3_ext.rearrange("p (b j) -> p b j", j=N_OUT))
            b1sb = wpool.tile([P, N_PAIR], f32)
            nc.scalar.dma_start(out=b1sb[:], in_=b1_ext[:])
            b2sb = wpool.tile([64, 4], f32)
            nc.scalar.dma_start(out=b2sb[:], in_=b2_ext[:])
            b3sb = wpool.tile([P, N_OUT], f32)
            nc.scalar.dma_start(out=b3sb[:], in_=b3_ext[:])

            static_xt = None
            if opts and opts.get("no_dma"):
                # micro-bench: load x once before the loop, body is compute-only
                static_xt = []
                b0 = 0
                for ch, nb in enumerate(opts.get("chunks", CHUNKS)):
                    sxt = wpool.tile([P, KT_ALL, nb], f16, name=f"sxt{ch}")
                    if opts.get("host_t"):
                        nc.sync.dma_start(
                            out=sxt[:],
                            in_=x_ext[:, KT_ALL * b0:KT_ALL * (b0 + nb)]
                            .rearrange("p (t b) -> p t b", b=nb))
                    else:
                        nc.sync.dma_start(out=sxt[:], in_=x_ext[b0:b0 + nb, :],
                                          transpose=True)
                    static_xt.append(sxt)
                    b0 += nb

            import contextlib
            loop_cm = tc.For_i(0, reps, 1) if reps > 1 else contextlib.nullcontext()
            with loop_cm:
                carry = {"pending": None, "env": None}
                for _ in range(unroll):
                    _kernel_body(nc, tc, locals(), has_bias, opts or {}, carry)
                if carry["pending"] is not None:
                    _l23(nc, carry["env"], has_bias,
                         (opts or {}).get("out_eng", "gpsimd"), *carry["pending"])
                    carry["pending"] = None

    nc.compile()
    return nc


def _kernel_body(nc, tc, env, has_bias, opts=None, carry=None):
    opts = opts or {}
    chunks = opts.get("chunks", CHUNKS)
    ring_mode = opts.get("rings", "dual")
    out_eng = opts.get("out_eng", "gpsimd")
    no_compute = opts.get("no_compute", False)   # micro-bench: DMAs only
    no_dma = opts.get("no_dma", False)           # micro-bench: compute only
    env["opts"] = opts
    defer_l23 = opts.get("defer_l23", False)
    cross_body = carry is not None and defer_l23 == 2
    pending = carry["pending"] if cross_body else None
    env["nbmax"] = max(chunks)
    x_ext = env["x_ext"]
    out_ext = env["out_ext"]
    w1sb, w2sb, w3sb = env["w1sb"], env["w2sb"], env["w3sb"]
    b1sb, b2sb, b3sb = env["b1sb"], env["b2sb"], env["b3sb"]
    xt_pool = env["xt_pool"]
    hp_pool, gt_pool, out_pool = env["hp_pool"], env["gt_pool"], env["out_pool"]
    ps1_pool, ps2_pool, ps3_pool = env["ps1_pool"], env["ps2_pool"], env["ps3_pool"]
    f16 = mybir.dt.float16
    f32 = mybir.dt.float32

    b0 = 0
    for ch, nb in enumerate(chunks):
        # one xbar transpose: x[b0:b0+nb, :] (DRAM, fp16) -> [128k, 48, nb].
        # Exact-size tile => contiguous SBUF destination; optionally alternate
        # the two HWDGE rings (SP / ACT) so two transposes can be in flight.
        engs = {"sync": nc.sync, "scalar": nc.scalar, "gpsimd": nc.gpsimd}
        load_rings = opts.get(
            "load_rings",
            ["sync"] if ring_mode == "single" else ["sync", "scalar"])
        ring = engs[load_rings[ch % len(load_rings)]]
        xt_parts = None
        if env.get("static_xt"):
            xt = env["static_xt"][ch]
        elif opts.get("host_t") and opts.get("csplit"):
            # one tile + one DMA per channel (or half-channel with csplit=2):
            # finer load granularity while keeping full-width matmuls (an L1
            # matmul never spans channels)
            sub = 2 if opts.get("csplit") == 2 else 1
            kt_piece = KT_CH // sub
            xt_parts = []
            for c in range(N_CH * sub):
                xc = xt_pool.tile([P, kt_piece, nb], f16, name=f"xtc{ch}c{c}")
                off = KT_ALL * b0 + c * kt_piece * nb
                ring.dma_start(
                    out=xc[:],
                    in_=x_ext[:, off:off + kt_piece * nb]
                    .rearrange("p (t b) -> p t b", b=nb))
                xt_parts.append(xc)
            env["kt_piece"] = kt_piece
            xt = None
        else:
            xt_t = xt_pool.tile([P, KT_ALL, nb], f16, name=f"xtc{ch}")
            xt = xt_t
            if opts.get("host_t"):
                ring.dma_start(
                    out=xt[:],
                    in_=x_ext[:, KT_ALL * b0:KT_ALL * (b0 + nb)]
                    .rearrange("p (t b) -> p t b", b=nb))
            else:
                ring.dma_start(out=xt[:], in_=x_ext[b0:b0 + nb, :],
                               transpose=True)

        if no_compute:
            # anti-DCE consumer: one tiny matmul + copy, then store
            xt0 = xt_parts[0][:, 0, :] if xt_parts is not None else xt[:, 0, :]
            ps_t = ps1_pool.tile([P, nb], f32, name="ncps")
            nc.tensor.matmul(ps_t[:], w1sb[:, 0, :], xt0,
                             start=True, stop=True)
            osb_t = out_pool.tile([P, nb // P, N_OUT], f32, name=f"osbc{ch}")
            nc.vector.tensor_copy(out=osb_t[:, 0, :128], in_=ps_t[:, :128])
            oeng = {"gpsimd": nc.gpsimd, "scalar": nc.scalar,
                    "sync": nc.sync}[out_eng]
            oeng.dma_start(
                out=out_ext[b0:b0 + nb, :].rearrange("(j p) n -> p j n", p=P),
                in_=osb_t[:],
            )
            b0 += nb
            continue

        # ---- layer 1: banded matmuls per window pair ----
        hps = {}
        for m in range(N_PAIR):
            ps_t = ps1_pool.tile([P, env["nbmax"]], f32, name="ps1t")
            ps = ps_t[:, :nb]
            mm_list = [(c, t) for c in range(N_CH) for t in _pair_tiles(m)]
            for i, (c, t) in enumerate(mm_list):
                if xt_parts is not None:
                    kt_piece = env["kt_piece"]
                    kg = c * KT_CH + t
                    rhs = xt_parts[kg // kt_piece][:, kg % kt_piece, :]
                else:
                    rhs = xt[:, c * KT_CH + t, :]
                nc.tensor.matmul(
                    ps[:],
                    w1sb[:, BLK_IDX[(m, c, t)], :],
                    rhs,
                    start=(i == 0),
                    stop=(i == len(mm_list) - 1),
                )
            hp_t = hp_pool.tile([P, env["nbmax"]], f16, name="hpt")
            hp = hp_t[:, :nb]
            if has_bias:
                nc.vector.tensor_scalar_add(hp[:], ps[:], b1sb[:, m:m + 1])
            elif opts.get("act_copies") and m % 2 == 1:
                nc.scalar.activation(hp[:], ps[:],
                                     mybir.ActivationFunctionType.Copy)
            else:
                nc.vector.tensor_copy(out=hp[:], in_=ps[:])
            hps[m] = hp

        if opts.get("l1_only"):
            # micro-bench: one tiny consumer per chunk so L1 isn't DCE'd
            osb_t = out_pool.tile([P, nb // P, N_OUT], f32, name=f"osbc{ch}")
            nc.vector.tensor_copy(out=osb_t[:, 0, :128], in_=hps[7][:, :128])
            oeng = {"gpsimd": nc.gpsimd, "scalar": nc.scalar,
                    "sync": nc.sync}[out_eng]
            oeng.dma_start(
                out=out_ext[b0:b0 + nb, :].rearrange("(j p) n -> p j n", p=P),
                in_=osb_t[:],
            )
        elif defer_l23:
            # emit chunk c-1's layers 2/3 AFTER chunk c's layer-1 matmuls so
            # the PE never waits on the DVE copies at a chunk boundary
            if pending is not None:
                _l23(nc, env, has_bias, out_eng, *pending)
            pending = (ch, nb, b0, hps)
        else:
            _l23(nc, env, has_bias, out_eng, ch, nb, b0, hps)
        b0 += nb
    if cross_body:
        # leave the last chunk's L2/3 pending past the next body's first L1
        carry["pending"] = pending
        carry["env"] = env
    elif pending is not None:
        _l23(nc, env, has_bias, out_eng, *pending)


def _l23(nc, env, has_bias, out_eng, ch, nb, b0, hps):
    out_ext = env["out_ext"]
    opts = env.get("opts") or {}
    w2sb, w3sb = env["w2sb"], env["w3sb"]
    b2sb, b3sb = env["b2sb"], env["b3sb"]
    gt_pool, out_pool = env["gt_pool"], env["out_pool"]
    ps2_pool, ps3_pool = env["ps2_pool"], env["ps3_pool"]
    if opts.get("ps23_merge"):
        ps3_pool = ps2_pool
    f16 = mybir.dt.float16
    f32 = mybir.dt.float32

    # ---- layer 2: 4 groups of 4 windows; two 64-wide groups share one
    # 128-partition PSUM tile so the PSUM->SBUF copy is full-width ----
    gt_t = gt_pool.tile([P, 2, env.get("nbmax", NB)], f16, name="gtt")
    gt = gt_t[:, :, :nb]
    for gp in range(2):              # group pair (2*gp, 2*gp+1)
        ps2_t = ps2_pool.tile([P, env.get("nbmax", NB)], f32, name="ps2t")
        ps2 = ps2_t[:, :nb]
        for half in range(2):
            g = 2 * gp + half
            lo = 64 * half
            for piece in range(2):
                nc.tensor.matmul(
                    ps2[lo:lo + 64],
                    w2sb[:, 2 * g + piece, :],
                    hps[2 * g + piece][:],
                    start=(piece == 0),
                    stop=(piece == 1),
                )
        if has_bias:
            for half in range(2):
                g = 2 * gp + half
                lo = 64 * half
                nc.vector.tensor_scalar_add(
                    gt[lo:lo + 64, gp], ps2[lo:lo + 64], b2sb[:, g:g + 1],
                )
        else:
            nc.vector.tensor_copy(out=gt[:, gp], in_=ps2[:])

    # ---- layer 3: back to batch-major ----
    osb_t = out_pool.tile([P, nb // P, N_OUT], f32, name=f"osbc{ch}")
    osb = osb_t
    for js in range(nb // P):
        ps3 = ps3_pool.tile([P, N_OUT], f32, name="ps3t")
        for piece in range(2):
            nc.tensor.matmul(
                ps3[:],
                gt[:, piece, js * P:(js + 1) * P],
                w3sb[:, piece, :],
                start=(piece == 0),
                stop=(piece == 1),
            )
        if has_bias:
            nc.vector.tensor_tensor(
                osb[:, js], ps3[:], b3sb[:], mybir.AluOpType.add,
            )
        else:
            nc.vector.tensor_copy(out=osb[:, js], in_=ps3[:])
    # SWDGE (gpsimd) store keeps both HWDGE rings free for transposes
    oeng = {"gpsimd": nc.gpsimd, "scalar": nc.scalar, "sync": nc.sync}[out_eng]
    oeng.dma_start(
        out=out_ext[b0:b0 + nb, :].rearrange("(j p) n -> p j n", p=P),
        in_=osb[:],
    )


_CACHED_NC = None


def _pack_x_host(x16_shard, chunks, csplit=False):
    """[B_SHARD, 6144] -> [128, 48*B_SHARD]: per chunk (and per channel when
    csplit), k-tile-major, partition-major so each DMA is one contiguous run
    per partition."""
    parts = []
    b0 = 0
    for nb in chunks:
        blk = x16_shard[b0:b0 + nb]                       # [nb, 6144]
        h = blk.T.reshape(KT_ALL, P, nb).transpose(1, 0, 2)   # [128, 48, nb]
        # csplit keeps channel blocks adjacent, which this layout already does
        # (k-tiles 0-15 = ch0, 16-31 = ch1, 32-47 = ch2)
        parts.append(h.reshape(P, KT_ALL * nb))
        b0 += nb
    return np.ascontiguousarray(np.concatenate(parts, axis=1))


def _prep_in_maps(x, W1, b1, W2, b2, W3, b3, opts=None):
    opts = opts or {}
    x16 = np.asarray(x, dtype=np.float16)
    w1sb, w2sb, w3sb, b1t, b2t, b3t = _pack_weights(W1, b1, W2, b2, W3, b3)
    in_maps = []
    for i in range(N_CORES):
        xs = x16[i * B_SHARD:(i + 1) * B_SHARD]
        if opts.get("host_t"):
            xs = _pack_x_host(xs, opts.get("chunks", CHUNKS))
        in_maps.append({
            "x": xs,
            "w1": w1sb,
            "w2": w2sb,
            "w3": w3sb,
            "b1t": b1t,
            "b2t": b2t,
            "b3t": b3t,
        })
    return in_maps


_CACHED_BIAS_NC = None

# best-measured configuration (see abtest batches): host pre-transposed x,
# plain single-queue loads, layers 2/3 deferred past the next chunk's layer 1
BEST_OPTS = {
    "host_t": True,
    "rings": "single",
    "chunks": [512, 512],
    "defer_l23": True,
    "csplit": True,
    # 6 L1 pair-PSUM buffers (vs 4/2/2): more accumulations in flight before
    # DVE copies gate bank recycling; L2/L3 single-buffered is off the PE
    # critical path thanks to defer_l23. Measured -2.3us in-batch.
    "ps1_bufs": 6,
    "ps2_bufs": 1,
    "ps3_bufs": 1,
}
BEST_UNROLL = 6  # For_i bodies per loop iteration in the timing harness


def kernel(x, W1, b1, W2, b2, W3, b3):
    global _CACHED_NC, _CACHED_BIAS_NC
    has_bias = bool(
        np.any(np.asarray(b1)) or np.any(np.asarray(b2)) or np.any(np.asarray(b3))
    )
    if has_bias:
        if _CACHED_BIAS_NC is None:
            _CACHED_BIAS_NC = build_kernel(has_bias=True, opts=BEST_OPTS)
        nc = _CACHED_BIAS_NC
    else:
        if _CACHED_NC is None:
            _CACHED_NC = build_kernel(opts=BEST_OPTS)
        nc = _CACHED_NC
    in_maps = _prep_in_maps(x, W1, b1, W2, b2, W3, b3, opts=BEST_OPTS)
    last_err = None
    for attempt in range(3):
        try:
            res = run_bass_kernel_spmd(nc, in_maps, core_ids=list(range(N_CORES)))
            break
        except Exception as e:  # transient device/axon failures
            last_err = e
            if attempt == 2:
                raise
            import time as _time
            _time.sleep(20.0)
    return np.concatenate([res.results[i]["out"] for i in range(N_CORES)], axis=0)

